# revision 9
# baseline (speedup 1.0000x reference)
"""AttGRU cell on 8 TRN2 NeuronCores.

Math (per reference):
    agg = einsum('ij,bj->bi', adj, x)                  # [B, N]
    r   = sigmoid(agg + h @ W_hr.T + b_hr)
    z   = sigmoid(agg + h @ W_hz.T + b_hz)
    n   = tanh(agg + r * (h @ W_hn.T + b_hn))
    out = (1 - z) * n + z * h

B=8, N=4096. Memory-bound: the four [N, N] f32 matrices (256 MB) dominate.

Sharding: row-shard adj/W_* over 8 cores (512 output features per core),
replicate x/h (tiny). Each core computes its 512 output columns; the host
concatenates. No collectives.

v2: gate-major weight streaming (adj -> W_hr -> W_hn -> W_hz) so each
gate's epilogue overlaps the next gate's DMA stream; only the z-gate tail
stays serial. tanh computed as 2*sigmoid(2u)-1 so ScalarE keeps a single
activation table (no mid-tail table reload).

v3: z gate streamed in two column halves (left fully, then right) so the
left half's sigmoid/blend/out-DMA overlap the right half's stream; agg is
folded into the z PSUM accumulators with a tiny identity matmul (removes
the agg-add from the tail); the final z slab is split into two DMAs so the
PE trails the last transfer by only a few chunks.

Per-core inputs (host-prepared):
  wall [9, 128, 5632] bf16 - gates adj, Whr, Whn: the sharded, transposed
       matrix as 33 contraction chunks of [128, 512] (chunk 32 is the bias
       row-chunk so biases ride the matmul), 11 chunks per DMA slab.
  wallz [2, 3, 128, 2816] bf16 - W_hz in column halves, 11 half-chunks
       ([128, 256]) per slab.
  vt   [128, 528] bf16 - stationary operand: [x.T | h.T] per chunk
       ([128, 16]); chunk 32 is [0 | ones-row] to activate the biases.
  hloc [8, 512] f32 - h column shard for the output blend.
  eye  [8, 8] f32 - identity, for folding agg into the z accumulators.

bf16 halves HBM traffic vs f32 and streams at 1 cycle/row on the PE
(f32 is 4 cycles/row); accumulation stays f32 in PSUM. rel err ~1.3e-3.
"""

from contextlib import ExitStack

import ml_dtypes
import numpy as np

import concourse.bass as bass
import concourse.tile as tile
from concourse import bacc, mybir
from concourse.bass_utils import run_bass_kernel_spmd

B = 8
N = 4096
NCORES = 8
S = N // NCORES          # 512 output cols per core
KC = 128                 # contraction chunk (PE partition dim)
NK = N // KC             # 32 data chunks
NKB = NK + 1             # +1 bias chunk
CHUNKS_PER_SLAB = 11     # 33 = 3 * 11
SLABS_PER_GATE = NKB // CHUNKS_PER_SLAB  # 3
SLABW = CHUNKS_PER_SLAB * S              # 5632
M2 = 2 * B               # 16: [x | h] stationary columns
ZH = S // 2              # 256: z-gate half width
ZSPLIT = 7               # final z slab: chunks 0:7, then 7:11 as 2nd DMA

BF16 = mybir.dt.bfloat16
F32 = mybir.dt.float32

_CACHED_NC = None


def _build():
    nc = bacc.Bacc(
        "TRN2",
        target_bir_lowering=False,
        debug=False,
        num_devices=NCORES,
    )
    wall = nc.dram_tensor(
        "wall", [3 * SLABS_PER_GATE, KC, SLABW], BF16, kind="ExternalInput"
    )
    wallz = nc.dram_tensor(
        "wallz", [2, SLABS_PER_GATE, KC, CHUNKS_PER_SLAB * ZH], BF16,
        kind="ExternalInput",
    )
    vt = nc.dram_tensor("vt", [KC, NKB * M2], BF16, kind="ExternalInput")
    hloc = nc.dram_tensor("hloc", [B, S], F32, kind="ExternalInput")
    eye = nc.dram_tensor("eye", [B, B], F32, kind="ExternalInput")
    out = nc.dram_tensor("out", [B, S], F32, kind="ExternalOutput")

    AF = mybir.ActivationFunctionType
    ALU = mybir.AluOpType

    with tile.TileContext(nc) as tc, ExitStack() as ctx:
        wpool = ctx.enter_context(tc.tile_pool(name="wall", bufs=3))
        cpool = ctx.enter_context(tc.tile_pool(name="const", bufs=1))
        ppool = ctx.enter_context(tc.tile_pool(name="acc", bufs=1, space="PSUM"))
        epool = ctx.enter_context(tc.tile_pool(name="epi", bufs=1))

        vt_sb = cpool.tile([KC, NKB * M2], BF16, tag="vt")
        nc.gpsimd.dma_start(vt_sb[:], vt[:])
        hloc_sb = cpool.tile([B, S], F32, tag="hloc")
        nc.gpsimd.dma_start(hloc_sb[:], hloc[:])
        eye_sb = cpool.tile([B, B], F32, tag="eye")
        nc.gpsimd.dma_start(eye_sb[:], eye[:])

        acc = [
            ppool.tile([B, S], F32, tag=f"acc{g}", name=f"acc{g}") for g in range(3)
        ]
        accz = [
            ppool.tile([B, ZH], F32, tag=f"accz{hf}", name=f"accz{hf}")
            for hf in range(2)
        ]

        # epilogue tiles, declared up front
        s_agg = epool.tile([B, S], F32, tag="sagg")
        t_r = epool.tile([B, S], F32, tag="tr")
        r_t = epool.tile([B, S], F32, tag="r")
        t_n = epool.tile([B, S], F32, tag="tn")
        t_n2 = epool.tile([B, S], F32, tag="tn2")
        sg_t = epool.tile([B, S], F32, tag="sg")
        n_t = epool.tile([B, S], F32, tag="n")
        d_t = epool.tile([B, S], F32, tag="d")
        z_t = epool.tile([B, S], F32, tag="z")
        zd_t = epool.tile([B, S], F32, tag="zd")
        o_t = epool.tile([B, S], F32, tag="o")

        def vt_x(k):
            return vt_sb[:, k * M2 : k * M2 + B]

        def vt_h(k):
            return vt_sb[:, k * M2 + B : (k + 1) * M2]

        # gates in stream order: 0=adj, 1=W_hr, 2=W_hn
        for g in range(3):
            for sl in range(SLABS_PER_GATE):
                wt = wpool.tile([KC, SLABW], BF16, tag="wt", name=f"wt{g}_{sl}")
                nc.sync.dma_start(wt[:], wall[g * SLABS_PER_GATE + sl])
                for c in range(CHUNKS_PER_SLAB):
                    k = sl * CHUNKS_PER_SLAB + c
                    nc.tensor.matmul(
                        acc[g][:, :],
                        vt_x(k) if g == 0 else vt_h(k),
                        wt[:, c * S : (c + 1) * S],
                        start=(k == 0),
                        stop=(k == NKB - 1),
                    )
            # per-gate epilogue; Tile starts each as soon as deps clear
            if g == 0:
                nc.vector.tensor_copy(s_agg[:], acc[0][:, :])
            elif g == 1:
                nc.vector.tensor_add(t_r[:], acc[1][:, :], s_agg[:])
                nc.scalar.activation(r_t[:], t_r[:], AF.Sigmoid)
            else:
                nc.vector.tensor_mul(t_n[:], acc[2][:, :], r_t[:])
                nc.vector.tensor_add(t_n2[:], t_n[:], s_agg[:])
                # tanh(u) = 2*sigmoid(2u) - 1 (keeps ACT on one table)
                nc.scalar.activation(sg_t[:], t_n2[:], AF.Sigmoid, scale=2.0)
                nc.vector.tensor_scalar(
                    n_t[:], sg_t[:], 2.0, 1.0, ALU.mult, ALU.subtract
                )
                nc.vector.tensor_sub(d_t[:], hloc_sb[:], n_t[:])

        # z gate, in column halves: left fully streamed, then right
        for hf in range(2):
            cols = slice(hf * ZH, (hf + 1) * ZH)
            # fold agg into the accumulator (group opener clears PSUM)
            nc.tensor.matmul(
                accz[hf][:, :], eye_sb[:, :], s_agg[:, cols], start=True, stop=False
            )
            for sl in range(SLABS_PER_GATE):
                wt = wpool.tile(
                    [KC, CHUNKS_PER_SLAB * ZH], BF16, tag="wtz", name=f"wtz{hf}_{sl}"
                )
                last = hf == 1 and sl == SLABS_PER_GATE - 1
                if last:
                    nc.sync.dma_start(
                        wt[:, : ZSPLIT * ZH], wallz[hf, sl][:, : ZSPLIT * ZH]
                    )
                    nc.sync.dma_start(
                        wt[:, ZSPLIT * ZH :], wallz[hf, sl][:, ZSPLIT * ZH :]
                    )
                else:
                    nc.sync.dma_start(wt[:], wallz[hf, sl])
                for c in range(CHUNKS_PER_SLAB):
                    k = sl * CHUNKS_PER_SLAB + c
                    nc.tensor.matmul(
                        accz[hf][:, :],
                        vt_h(k),
                        wt[:, c * ZH : (c + 1) * ZH],
                        start=False,
                        stop=(k == NKB - 1),
                    )
            nc.scalar.activation(z_t[:, cols], accz[hf][:, :], AF.Sigmoid)
            nc.vector.tensor_mul(zd_t[:, cols], z_t[:, cols], d_t[:, cols])
            nc.vector.tensor_add(o_t[:, cols], zd_t[:, cols], n_t[:, cols])
            nc.sync.dma_start(out[:, cols], o_t[:, cols])

    nc.compile()
    return nc


def _get_nc():
    global _CACHED_NC
    if _CACHED_NC is None:
        _CACHED_NC = _build()
    return _CACHED_NC


def make_in_maps(x, h, adj, W_hr, b_hr, W_hz, b_hz, W_hn, b_hn):
    bf = ml_dtypes.bfloat16
    x = np.asarray(x, np.float32)
    h = np.asarray(h, np.float32)
    adj = np.asarray(adj, np.float32)
    W_hr = np.asarray(W_hr, np.float32)
    W_hz = np.asarray(W_hz, np.float32)
    W_hn = np.asarray(W_hn, np.float32)
    b_hr = np.asarray(b_hr, np.float32)
    b_hz = np.asarray(b_hz, np.float32)
    b_hn = np.asarray(b_hn, np.float32)

    vt_full = np.zeros((NKB * KC, M2), np.float32)
    vt_full[:N, :B] = x.T
    vt_full[:N, B:] = h.T
    vt_full[N, B:] = 1.0  # bias-chunk ones row (h side only)
    vt_packed = np.ascontiguousarray(
        vt_full.reshape(NKB, KC, M2).transpose(1, 0, 2).reshape(KC, NKB * M2)
    ).astype(bf)

    in_maps = []
    for s in range(NCORES):
        rs, re = s * S, (s + 1) * S
        # stream order: adj, W_hr, W_hn (z last, packed separately)
        slabs = []
        for W, b in ((adj, None), (W_hr, b_hr), (W_hn, b_hn)):
            gm = np.zeros((NKB * KC, S), np.float32)
            gm[:N] = W[rs:re].T
            if b is not None:
                gm[N] = b[rs:re]
            slabs.append(
                gm.reshape(SLABS_PER_GATE, CHUNKS_PER_SLAB, KC, S)
                .transpose(0, 2, 1, 3)
                .reshape(SLABS_PER_GATE, KC, SLABW)
            )
        wallp = np.ascontiguousarray(np.concatenate(slabs, axis=0)).astype(bf)

        gm = np.zeros((NKB * KC, S), np.float32)
        gm[:N] = W_hz[rs:re].T
        gm[N] = b_hz[rs:re]
        # [2 halves, 3 slabs, 128, 11*256]
        wallzp = np.ascontiguousarray(
            gm.reshape(NKB * KC, 2, ZH)
            .transpose(1, 0, 2)
            .reshape(2, SLABS_PER_GATE, CHUNKS_PER_SLAB, KC, ZH)
            .transpose(0, 1, 3, 2, 4)
            .reshape(2, SLABS_PER_GATE, KC, CHUNKS_PER_SLAB * ZH)
        ).astype(bf)

        in_maps.append(
            {
                "wall": wallp,
                "wallz": wallzp,
                "vt": vt_packed,
                "hloc": np.ascontiguousarray(h[:, rs:re]),
                "eye": np.eye(B, dtype=np.float32),
            }
        )
    return in_maps


def run(in_maps, trace=False, **kw):
    nc = _get_nc()
    return run_bass_kernel_spmd(
        nc, in_maps, core_ids=list(range(NCORES)), trace=trace, **kw
    )


def kernel(x, h, adj, W_hr, b_hr, W_hz, b_hz, W_hn, b_hn):
    in_maps = make_in_maps(x, h, adj, W_hr, b_hr, W_hz, b_hz, W_hn, b_hn)
    res = run(in_maps)
    return np.concatenate(
        [np.asarray(res.results[s]["out"]) for s in range(NCORES)], axis=1
    )
